# revision 1
# baseline (speedup 1.0000x reference)
"""2-layer GAT (GATConv x2, mean over 4 heads) on 8 Trainium2 NeuronCores.

Strategy (dst-segment parallel, gather-instruction-minimal):
  - Host: relabel nodes by greedy in-degree bin-packing into 400 tiles of
    128 (50 tiles per core) so every tile has near-equal edge count ->
    uniform K chunks of 128 edges per tile with ~no padding waste.  Each
    core owns all edges of its 50 tiles, so segment softmax never crosses
    cores (no cross-core reduction needed).
  - Self loops are not in the edge list: chunk 0 of each tile gathers the
    tile's own 128 node rows (dst_local = iota), which simultaneously
    provides the per-tile a_d vector (cols 516:520 of the gathered rows).
  - G rows (bf16): [xh(512) | a_s(4) | a_d(4)], stored 4-way interleaved
    (G row of node (T,p) = (T>>2)*512 + 4p + (T&3)) so phase-A stores move
    4 rows per DMA descriptor.  Gather indices are pre-mapped on host.
  - Phase B per 128-edge chunk: indirect-DMA gather of G rows (one row per
    edge/partition), mask[e,p] = (dstl[e]==p), maskT via PE transpose,
    per-edge a_d via maskT.T @ ad_tile, ex = exp(lrelu(a_s+a_d)) (softmax
    max-shift skipped: alpha is O(1) and softmax is shift-invariant),
    msg = ex_h * xh_h, then out += mask.T @ msg and den += mask.T @ ex as
    PE matmuls accumulating in fp32 PSUM.  h = sum_h out_h/(4 den_h) + b.
  - h tiles are PE-transposed, AllGather'd in bf16, and consumed as
    ready-made lhsT slabs by layer-2's phase A.
"""

import numpy as np

P = 128
NCORES = 8

_CACHE = {}


def _pack_nodes(in_deg, NT):
    """Greedy largest-first packing of nodes into NT tiles of 128 slots."""
    import heapq
    n = len(in_deg)
    order = np.argsort(-in_deg, kind="stable")
    heap = [(0, t) for t in range(NT)]
    heapq.heapify(heap)
    newid = np.empty(NT * P, np.int64)
    fill = np.zeros(NT, np.int32)
    counts = np.zeros(NT, np.int64)
    for node in order:
        cnt, t = heapq.heappop(heap)
        newid[node] = t * P + fill[t]
        fill[t] += 1
        counts[t] = cnt + in_deg[node]
        if fill[t] < P:
            heapq.heappush(heap, (int(counts[t]), t))
    pad = n
    for t in range(NT):
        while fill[t] < P:
            newid[pad] = t * P + fill[t]
            fill[t] += 1
            pad += 1
    return newid, counts


def _rho(n):
    """node-id -> G-row-id: 4-way interleave within groups of 4 tiles."""
    return ((n >> 9) << 9) | ((n & 127) << 2) | ((n >> 7) & 3)


def _host_prep(x, edge_index, W1, att_src1, att_dst1, b1, W2, att_src2,
               att_dst2, b2):
    bf = np.float16

    N, IN_F = x.shape
    HEADS, HID = att_src1.shape
    OUT_F = att_src2.shape[1]
    TPC = -(-int(N * 1.02) // (NCORES * P))   # ~2% slack for packing
    NT = NCORES * TPC
    NP_ = NT * P
    assert NP_ >= N

    src0 = np.asarray(edge_index[0], np.int64)
    dst0 = np.asarray(edge_index[1], np.int64)
    in_deg = np.bincount(dst0, minlength=N)
    newid, counts = _pack_nodes(in_deg, NT)
    K = max(1, int(np.max(-(-counts // P))))    # real-edge chunks per tile

    src = newid[src0]
    dst = newid[dst0]
    order = np.argsort(dst, kind="stable")
    srcs = src[order]
    dsts = dst[order]
    E = srcs.shape[0]

    tile_of_edge = dsts // P
    cnt2 = np.bincount(tile_of_edge, minlength=NT)
    assert int(np.max(cnt2)) <= K * P
    bounds = np.concatenate([[0], np.cumsum(cnt2)])
    within = np.arange(E) - bounds[tile_of_edge]

    KT = K + 1                                  # + self chunk 0
    SRCp = np.zeros((NT, KT * P), np.int64)
    DSTLp = np.full((NT, KT * P), -1.0, np.float32)
    SRCp[:, 0:P] = (np.arange(NT)[:, None] * P + np.arange(P)[None, :])
    DSTLp[:, 0:P] = np.arange(P, dtype=np.float32)[None, :]
    SRCp[tile_of_edge, P + within] = srcs
    DSTLp[tile_of_edge, P + within] = (dsts - tile_of_edge * P).astype(
        np.float32)
    SRCp = _rho(SRCp).astype(np.int32)

    def per_core(arr, npdt):
        out = []
        for i in range(NCORES):
            a = arr[i * TPC:(i + 1) * TPC].reshape(TPC * KT, P)
            out.append(np.ascontiguousarray(a.T).astype(npdt))
        return out

    SRC_cores = per_core(SRCp, np.int32)
    DSTL_cores = per_core(DSTLp, np.float32)

    def wcat(W, att_s, att_d, ch):
        As = np.zeros((HEADS * ch, HEADS), np.float32)
        Ad = np.zeros((HEADS * ch, HEADS), np.float32)
        for h in range(HEADS):
            As[h * ch:(h + 1) * ch, h] = att_s[h]
            Ad[h * ch:(h + 1) * ch, h] = att_d[h]
        WT = W.T.astype(np.float32)
        return np.concatenate([WT, WT @ As, WT @ Ad], axis=1)

    W1cat = np.ascontiguousarray(wcat(W1, att_src1, att_dst1, HID)).astype(bf)
    W2cat = np.ascontiguousarray(wcat(W2, att_src2, att_dst2, OUT_F)).astype(bf)

    x_new = np.zeros((NP_, IN_F), np.float32)
    x_new[newid[:N]] = x
    xT = np.ascontiguousarray(x_new.T).astype(bf)

    IOTA = np.broadcast_to(np.arange(P, dtype=np.float32), (P, P)).copy()
    IDENT = np.eye(P, dtype=bf)
    B1bc = np.broadcast_to(b1.astype(np.float32), (P, HID)).copy()
    B2bc = np.broadcast_to(b2.astype(np.float32), (P, OUT_F)).copy()

    shapes = dict(N=N, IN_F=IN_F, HEADS=HEADS, HID=HID, OUT_F=OUT_F,
                  NP=NP_, NT=NT, TPC=TPC, K=KT)
    shared = dict(xT=xT, W1cat=W1cat, W2cat=W2cat, IOTA=IOTA, IDENT=IDENT,
                  B1bc=B1bc, B2bc=B2bc, newid=newid)
    percore = [dict(SRC=SRC_cores[i], DSTL=DSTL_cores[i])
               for i in range(NCORES)]
    return shapes, shared, percore


def _build(s):
    import concourse.bass as bass
    import concourse.mybir as mybir
    import concourse.tile as tile
    from concourse import bacc

    f32 = mybir.dt.float32
    bf16 = mybir.dt.float16
    i32 = mybir.dt.int32
    HEADS, HID, OUT_F, IN_F = s["HEADS"], s["HID"], s["OUT_F"], s["IN_F"]
    NP_, NT, TPC, KT = s["NP"], s["NT"], s["TPC"], s["K"]
    NH = HEADS * HID                    # 512
    GW = NH + 2 * HEADS                 # 520
    NCH = TPC * KT
    KC1 = IN_F // P
    AluOp = mybir.AluOpType
    Act = mybir.ActivationFunctionType

    nc = bacc.Bacc("TRN2", target_bir_lowering=False, debug=False,
                   num_devices=NCORES)

    t_xT = nc.dram_tensor("xT", [IN_F, NP_], bf16, kind="ExternalInput")
    t_w1 = nc.dram_tensor("W1cat", [IN_F, GW], bf16, kind="ExternalInput")
    t_w2 = nc.dram_tensor("W2cat", [HID, GW], bf16, kind="ExternalInput")
    t_iota = nc.dram_tensor("IOTA", [P, P], f32, kind="ExternalInput")
    t_ident = nc.dram_tensor("IDENT", [P, P], bf16, kind="ExternalInput")
    t_b1 = nc.dram_tensor("B1bc", [P, HID], f32, kind="ExternalInput")
    t_b2 = nc.dram_tensor("B2bc", [P, OUT_F], f32, kind="ExternalInput")
    t_src = nc.dram_tensor("SRC", [P, NCH], i32, kind="ExternalInput")
    t_dstl = nc.dram_tensor("DSTL", [P, NCH], f32, kind="ExternalInput")
    t_out = nc.dram_tensor("out", [TPC * P, OUT_F], f32, kind="ExternalOutput")

    with tile.TileContext(nc) as tc:
        with tc.tile_pool(name="const", bufs=1) as constp, \
             tc.tile_pool(name="dram", bufs=1, space="DRAM") as dramp, \
             tc.tile_pool(name="slab", bufs=2) as slabp, \
             tc.tile_pool(name="stage", bufs=3) as stagep, \
             tc.tile_pool(name="gat", bufs=6) as gatp, \
             tc.tile_pool(name="msk", bufs=6) as mskp, \
             tc.tile_pool(name="small", bufs=8) as smallp, \
             tc.tile_pool(name="accs", bufs=3) as accp:

            G1 = dramp.tile([NP_, GW], bf16, name="G1")
            G2 = dramp.tile([NP_, GW], bf16, name="G2")
            hT_sh = dramp.tile([1, P * TPC * P], bf16, name="hT_sh")
            hT_full = dramp.tile([NCORES, P * TPC * P], bf16, name="hT_full",
                                 addr_space="Shared")

            iota_sb = constp.tile([P, P], f32, name="iota_sb")
            nc.sync.dma_start(out=iota_sb[:], in_=t_iota[:, :])
            ident_sb = constp.tile([P, P], bf16, name="ident_sb")
            nc.sync.dma_start(out=ident_sb[:], in_=t_ident[:, :])
            b1_sb = constp.tile([P, HID], f32, name="b1_sb")
            nc.sync.dma_start(out=b1_sb[:], in_=t_b1[:, :])
            b2_sb = constp.tile([P, OUT_F], f32, name="b2_sb")
            nc.sync.dma_start(out=b2_sb[:], in_=t_b2[:, :])
            w1_sb = []
            for k in range(KC1):
                w1k = constp.tile([P, GW], bf16, name=f"w1_sb{k}")
                nc.sync.dma_start(out=w1k[:], in_=t_w1[k * P:(k + 1) * P, :])
                w1_sb.append(w1k)
            w2_sb = constp.tile([P, GW], bf16, name="w2_sb")
            nc.sync.dma_start(out=w2_sb[:], in_=t_w2[:, :])
            src_sb = constp.tile([P, NCH], i32, name="src_sb")
            nc.sync.dma_start(out=src_sb[:], in_=t_src[:, :])
            dstl_sb = constp.tile([P, NCH], f32, name="dstl_sb")
            nc.sync.dma_start(out=dstl_sb[:], in_=t_dstl[:, :])

            def phase_a(G, w_rhs, lhsT_provider):
                kc = len(w_rhs)
                with tc.tile_pool(name="psA", bufs=3, space="PSUM") as psA:
                    for T0 in range(0, NT, 4):
                        stg = stagep.tile([P, 4, GW], bf16, name="stg")
                        for q in range(4):
                            lhsTs = lhsT_provider(T0 + q)
                            ps = psA.tile([P, GW], f32, name="aps")
                            for k in range(kc):
                                nc.tensor.matmul(
                                    ps[:, 0:NH], lhsT=lhsTs[k],
                                    rhs=w_rhs[k][:, 0:NH],
                                    start=(k == 0), stop=(k == kc - 1))
                                nc.tensor.matmul(
                                    ps[:, NH:GW], lhsT=lhsTs[k],
                                    rhs=w_rhs[k][:, NH:GW],
                                    start=(k == 0), stop=(k == kc - 1))
                            nc.vector.tensor_copy(stg[:, q, :], ps[:, :])
                        dst = G[T0 * P:(T0 + 4) * P, :].rearrange(
                            "(p q) w -> p q w", q=4)
                        nc.sync.dma_start(out=dst, in_=stg[:])

            def make_l1_provider():
                SLAB = 2048                      # 16 tiles per slab
                while NP_ % SLAB:
                    SLAB -= 512
                cache = {}

                def provider(T):
                    sl = T * P // SLAB
                    if sl not in cache:
                        tiles = []
                        for k in range(KC1):
                            t_sl = slabp.tile([P, SLAB], bf16, name=f"xsl{k}")
                            nc.sync.dma_start(
                                out=t_sl[:],
                                in_=t_xT[k * P:(k + 1) * P,
                                         sl * SLAB:(sl + 1) * SLAB])
                            tiles.append(t_sl)
                        cache.clear()
                        cache[sl] = tiles
                    off = T * P - sl * SLAB
                    return [t[:, off:off + P] for t in cache[sl]]

                return provider

            def make_l2_provider():
                # hT_full viewed [p, core, col]; slab = 1280 cols (10 tiles)
                v = hT_full[:, :].rearrange("o (p c) -> p o c", p=P)
                per_core_cols = TPC * P          # 6400
                t10 = 10
                while TPC % t10:
                    t10 -= 1
                SLAB = t10 * P
                cache = {}

                def provider(T):
                    col = T * P
                    o, cc = col // per_core_cols, col % per_core_cols
                    sl = (o, cc // SLAB)
                    if sl not in cache:
                        t_sl = slabp.tile([P, SLAB], bf16, name="hsl")
                        nc.sync.dma_start(
                            out=t_sl[:],
                            in_=v[:, sl[0],
                                  sl[1] * SLAB:(sl[1] + 1) * SLAB])
                        cache.clear()
                        cache[sl] = t_sl
                    off = cc - sl[1] * SLAB
                    return [cache[sl][:, off:off + P]]

                return provider

            def phase_b(G, bbc_sb, writer):
              with tc.tile_pool(name="psB", bufs=2, space="PSUM") as psB:
                for t in range(TPC):
                    out_ps = psB.tile([P, NH], f32, name="outps")
                    den_ps = psB.tile([P, HEADS], f32, name="denps")
                    ad_tile = smallp.tile([P, HEADS], bf16, name="ad_tile")
                    for c in range(KT):
                        ci = t * KT + c
                        first, last = (c == 0), (c == KT - 1)
                        g = gatp.tile([P, GW], bf16, name="g")
                        nc.gpsimd.indirect_dma_start(
                            out=g[:], out_offset=None, in_=G[:, :],
                            in_offset=bass.IndirectOffsetOnAxis(
                                ap=src_sb[:, ci:ci + 1], axis=0))
                        if first:
                            nc.vector.tensor_copy(ad_tile[:],
                                                  g[:, NH + HEADS:GW])
                        mask = mskp.tile([P, P], bf16, name="mask")
                        nc.vector.tensor_scalar(
                            out=mask[:], in0=iota_sb[:],
                            scalar1=dstl_sb[:, ci:ci + 1], scalar2=None,
                            op0=AluOp.is_equal)
                        mtp = psB.tile([P, P], bf16, name="mtp")
                        nc.tensor.transpose(mtp[:], mask[:], ident_sb[:])
                        maskT = mskp.tile([P, P], bf16, name="maskT")
                        nc.vector.tensor_copy(maskT[:], mtp[:])
                        adpe = psB.tile([P, HEADS], f32, name="adpe")
                        nc.tensor.matmul(adpe[:, :], lhsT=maskT[:],
                                         rhs=ad_tile[:], start=True,
                                         stop=True)
                        asf = smallp.tile([P, HEADS], f32, name="asf")
                        nc.vector.tensor_copy(asf[:], g[:, NH:NH + HEADS])
                        ex = smallp.tile([P, HEADS], f32, name="ex")
                        nc.vector.tensor_tensor(out=ex[:], in0=asf[:],
                                                in1=adpe[:], op=AluOp.add)
                        t2 = smallp.tile([P, HEADS], f32, name="t2")
                        nc.vector.tensor_scalar_mul(t2[:], ex[:], 0.2)
                        nc.vector.tensor_tensor(out=ex[:], in0=ex[:],
                                                in1=t2[:], op=AluOp.max)
                        nc.scalar.activation(out=ex[:], in_=ex[:],
                                             func=Act.Exp)
                        msgex = mskp.tile([P, NH + HEADS], bf16, name="msgex")
                        nc.vector.tensor_copy(msgex[:, NH:NH + HEADS], ex[:])
                        for h in range(HEADS):
                            nc.vector.tensor_scalar_mul(
                                msgex[:, h * HID:(h + 1) * HID],
                                g[:, h * HID:(h + 1) * HID],
                                ex[:, h:h + 1])
                        nc.tensor.matmul(out_ps[:, :], lhsT=mask[:],
                                         rhs=msgex[:, 0:NH],
                                         start=first, stop=last)
                        nc.tensor.matmul(den_ps[:, :], lhsT=mask[:],
                                         rhs=msgex[:, NH:NH + HEADS],
                                         start=first, stop=last)
                    den4 = smallp.tile([P, HEADS], f32, name="den4")
                    nc.vector.tensor_scalar(
                        out=den4[:], in0=den_ps[:], scalar1=float(HEADS),
                        scalar2=float(HEADS) * 1e-16, op0=AluOp.mult,
                        op1=AluOp.add)
                    rec = smallp.tile([P, HEADS], f32, name="rec")
                    nc.vector.reciprocal(rec[:], den4[:])
                    acc = accp.tile([P, HID], f32, name="acc")
                    tmp = accp.tile([P, HID], f32, name="tmpacc")
                    nc.vector.tensor_scalar_mul(acc[:], out_ps[:, 0:HID],
                                                rec[:, 0:1])
                    for h in range(1, HEADS):
                        nc.vector.tensor_scalar_mul(
                            tmp[:], out_ps[:, h * HID:(h + 1) * HID],
                            rec[:, h:h + 1])
                        nc.vector.tensor_tensor(out=acc[:], in0=acc[:],
                                                in1=tmp[:], op=AluOp.add)
                    nc.vector.tensor_tensor(out=acc[:], in0=acc[:],
                                            in1=bbc_sb[:], op=AluOp.add)
                    writer(t, acc, psB)

            hT_shv = hT_sh[:, :].rearrange("o (p c) -> p (o c)", p=P)

            def write_h(t, acc, psB):
                accb = accp.tile([P, HID], bf16, name="accb")
                nc.vector.tensor_copy(accb[:], acc[:])
                tp = psB.tile([P, P], bf16, name="mtp")
                nc.tensor.transpose(tp[:], accb[:], ident_sb[:])
                hTt = accp.tile([P, P], bf16, name="hTt")
                nc.vector.tensor_copy(hTt[:], tp[:])
                nc.sync.dma_start(out=hT_shv[:, t * P:(t + 1) * P],
                                  in_=hTt[:])

            def write_out(t, acc, psB):
                nc.sync.dma_start(out=t_out[t * P:(t + 1) * P, :], in_=acc[:])

            import os
            PH = int(os.environ.get("KPH", "5"))
            with nc.named_scope("phA1"):
                phase_a(G1, w1_sb, make_l1_provider())
            if PH >= 2:
                with nc.named_scope("phB1"):
                    phase_b(G1, b1_sb, write_h)
            if PH >= 3:
                with nc.named_scope("phAG"):
                    nc.gpsimd.collective_compute(
                        "AllGather", AluOp.bypass,
                        replica_groups=[list(range(NCORES))],
                        ins=[hT_sh[:].opt()], outs=[hT_full[:].opt()])
            if PH >= 4:
                with nc.named_scope("phA2"):
                    phase_a(G2, [w2_sb], make_l2_provider())
            if PH >= 5:
                with nc.named_scope("phB2"):
                    phase_b(G2, b2_sb, write_out)

    nc.compile()
    return nc


def _get_nc(s):
    key = tuple(sorted(s.items()))
    if key not in _CACHE:
        _CACHE[key] = _build(s)
    return _CACHE[key]


def _in_maps(shared, percore):
    maps = []
    for i in range(NCORES):
        maps.append({"xT": shared["xT"], "W1cat": shared["W1cat"],
                     "W2cat": shared["W2cat"], "IOTA": shared["IOTA"],
                     "IDENT": shared["IDENT"], "B1bc": shared["B1bc"],
                     "B2bc": shared["B2bc"], "SRC": percore[i]["SRC"],
                     "DSTL": percore[i]["DSTL"]})
    return maps


def kernel(**inputs):
    from concourse import bass_utils

    x = np.asarray(inputs["x"], dtype=np.float32)
    edge_index = np.asarray(inputs["edge_index"])
    args = (x, edge_index,
            np.asarray(inputs["W1"], np.float32),
            np.asarray(inputs["att_src1"], np.float32),
            np.asarray(inputs["att_dst1"], np.float32),
            np.asarray(inputs["b1"], np.float32),
            np.asarray(inputs["W2"], np.float32),
            np.asarray(inputs["att_src2"], np.float32),
            np.asarray(inputs["att_dst2"], np.float32),
            np.asarray(inputs["b2"], np.float32))
    shapes, shared, percore = _host_prep(*args)
    nc = _get_nc(shapes)
    res = bass_utils.run_bass_kernel_spmd(nc, _in_maps(shared, percore),
                                          core_ids=list(range(NCORES)))
    out_cat = np.concatenate(
        [res.results[i]["out"] for i in range(NCORES)], axis=0)
    out = out_cat[shared["newid"][:shapes["N"]]]
    return np.ascontiguousarray(out, dtype=np.float32)

